# revision 35
# baseline (speedup 1.0000x reference)
"""Vocab-parallel AdvSmax loss kernel for 8 TRN2 NeuronCores.

Strategy (tensor parallel over vocab, per sharding hint):
  - Each core owns a contiguous vocab shard of dec_w/dec_b/enc_w and computes
    its slice of logits = h @ dec_w.T + dec_b with a bf16 matmul (K extended by
    one "ones" row so the bias rides in the matmul).
  - The adversarial noise term only touches element (i, targets[i]) of the
    logits; all rows i whose target falls in a core's shard are that core's
    "hits".  The noise value (and the scatter-of-last-duplicate semantics) is
    computed exactly in f32 from indirect row gathers of x / enc_w / dec_w --
    a tiny side channel (~2 MB) instead of touching the 450 MB logits.
  - log_softmax normalization: per-row shift (numerically safe against huge
    positive noise) agreed via one AllReduce, then per-row sum of exp(l-shift)
    via an AllReduce per row-chunk; output = l - (shift + ln(s)).
  - Hit elements of the output are patched at the end (second TileContext so
    the indirect scatter doesn't inherit conservative WAW deps against the
    main output DMAs) with the exact f32 value.

Hardware constraints honored throughout:
  - every DMA instruction may carry at most ONE semaphore wait, so producers
    of any DMA's inputs are funneled through a single engine (DVE copies of
    offset tiles), collective->queue deps are flushed with broadcast dummy
    reads, and zero-init buffers arrive as pre-zeroed input params;
  - indirect DMA offsets are (128, 1): one offset per partition, the free
    dim of the other side is the per-offset row.
"""

from dataclasses import dataclass, field

import numpy as np

import concourse.bacc as bacc
import concourse.bass as bass
import concourse.mybir as mybir
import concourse.tile as tile
from concourse.bass_utils import run_bass_kernel_spmd

f32 = mybir.dt.float32
bf16 = mybir.dt.bfloat16
i32 = mybir.dt.int32
AF = mybir.ActivationFunctionType
AL = mybir.AluOpType

ALPHA = 0.2
EPS = 1e-8
SHIFT0 = 8.0  # baseline exp shift; base logits are <~7 for this problem
PAD_BIAS = -10000.0  # bias for padded vocab columns -> exp underflows to 0


@dataclass
class Cfg:
    N: int = 2240          # rows (tokens)
    D: int = 400           # hidden dim
    V: int = 50257         # vocab
    NC: int = 8            # cores
    MT: int = 512          # matmul free-dim tile
    PSG: int = 3           # matmul col-tiles grouped per PSUM tile
    LDP: int = 4           # load pieces per k-chunk of dwT
    LGB: int = 2           # buffers per logits row-tile slot
    EXPW: int = 4          # exp instructions per row-tile
    FINLAG: int = 1        # chunks between compute and finish emission
    chunk: tuple = ()      # row-tiles per chunk; default computed
    act_frac: float = 0.25 # fraction of final-pass row-tiles on ACT engine

    NP: int = field(init=False)
    RT: int = field(init=False)
    K: int = field(init=False)
    KC: int = field(init=False)
    SW: int = field(init=False)
    VP: int = field(init=False)
    CT: int = field(init=False)

    def __post_init__(self):
        self.NP = ((self.N + 127) // 128) * 128
        self.RT = self.NP // 128
        self.K = self.D + 1
        self.KC = (self.K + 127) // 128
        self.SW = (self.V + self.NC - 1) // self.NC
        self.VP = ((self.SW + self.MT - 1) // self.MT) * self.MT
        self.CT = self.VP // self.MT
        if not self.chunk:
            per = 3 if self.RT > 3 else self.RT
            ch = []
            left = self.RT
            while left > 0:
                ch.append(min(per, left))
                left -= per
            self.chunk = tuple(ch)
        assert sum(self.chunk) == self.RT


def build(cfg: Cfg, maxh: int):
    """Build the SPMD Bass graph. maxh = padded per-core hit count (mult of 128)."""
    c = cfg
    HT = maxh // 128
    nc = bacc.Bacc(num_devices=c.NC)
    groups = [list(range(c.NC))]

    xT = nc.declare_dram_parameter("xT", [c.K, c.NP], bf16, isOutput=False)
    xr = nc.declare_dram_parameter("x", [c.N, c.D], f32, isOutput=False)
    dwT = nc.declare_dram_parameter("dwT", [c.K, c.SW], bf16, isOutput=False)
    dw = nc.declare_dram_parameter("dw", [c.SW, c.D], f32, isOutput=False)
    ew = nc.declare_dram_parameter("ew", [c.SW, c.D], f32, isOutput=False)
    db = nc.declare_dram_parameter("db", [c.SW, 1], f32, isOutput=False)
    hh = nc.declare_dram_parameter("hh", [maxh, 1], i32, isOutput=False)
    hp = nc.declare_dram_parameter("hp", [maxh, 1], i32, isOutput=False)
    htl = nc.declare_dram_parameter("htl", [maxh, 1], i32, isOutput=False)
    hoff = nc.declare_dram_parameter("hoff", [maxh, 1], i32, isOutput=False)
    # pre-zeroed per-row staging buffers (inputs, so no zeroing DMA is needed)
    exd_t = [
        nc.declare_dram_parameter(f"exd{t}", [c.NP, 1], f32, isOutput=False)
        for t in range(HT)
    ]
    dsh_t = [
        nc.declare_dram_parameter(f"dsh{t}", [c.NP, 1], f32, isOutput=False)
        for t in range(HT)
    ]
    out_ext = nc.declare_dram_parameter("out", [c.N, c.SW], f32, isOutput=True)

    # internal DRAM scratch
    lpdh = nc.dram_tensor("lpdh", [HT * 128, 1], f32)   # per-hit l+delta
    exd = nc.dram_tensor("exd", [c.NP, 1], f32)         # per-row extras (local)
    exg = nc.dram_tensor("exg", [c.NP, 1], f32, addr_space="Shared")
    nlzd = nc.dram_tensor("nlzd", [c.NP, 1], f32)       # -logZ bounce
    ccin = [nc.dram_tensor(f"ccin{i}", [128, sz], f32) for i, sz in enumerate(c.chunk)]
    ccout = [
        nc.dram_tensor(f"ccout{i}", [128, sz], f32, addr_space="Shared")
        for i, sz in enumerate(c.chunk)
    ]

    # DRAM view: element (m*128+p) laid out as [p, m] for SBUF (128, M) vectors
    def pm(t, m):
        return t[: m * 128].rearrange("(m p) o -> p (m o)", p=128)

    out_flat = out_ext[:].rearrange("n (v o) -> (n v) o", o=1)
    ioa = bass.IndirectOffsetOnAxis
    kc_last = c.KC - 1
    pr_last = c.K - 128 * kc_last

    with tile.TileContext(nc) as tc:
        with (
            tc.tile_pool(name="persist", bufs=1) as pp,
            tc.tile_pool(name="es", bufs=2) as esp,
            tc.tile_pool(name="psum", bufs=2, space="PSUM") as psp,
        ):
            # ---------------- noise side-channel (scheduled first) -----------
            hp_ctx = tc.high_priority()
            hp_ctx.__enter__()
            ihx = pp.tile([128, HT], i32, tag="ihx", name="ihx")
            ipx = pp.tile([128, HT], i32, tag="ipx", name="ipx")
            itx = pp.tile([128, HT], i32, tag="itx", name="itx")
            nc.sync.dma_start(out=ihx[:], in_=pm(hh, HT))
            nc.sync.dma_start(out=ipx[:], in_=pm(hp, HT))
            nc.sync.dma_start(out=itx[:], in_=pm(htl, HT))
            # DVE copy of hit-row offsets: scatters whose offsets AND values
            # are DVE-produced carry a single (DVE) wait.
            ihc = pp.tile([128, HT], i32, tag="ihc", name="ihc")
            nc.vector.tensor_copy(out=ihc[:], in_=ihx[:])

            GA, GB, GW, GD, GBV = [], [], [], [], []
            for t in range(HT):
                ga = pp.tile([128, c.D], f32, tag=f"ga{t}", name=f"ga{t}")
                gb = pp.tile([128, c.D], f32, tag=f"gb{t}", name=f"gb{t}")
                gw = pp.tile([128, c.D], f32, tag=f"gw{t}", name=f"gw{t}")
                gd = pp.tile([128, c.D], f32, tag=f"gd{t}", name=f"gd{t}")
                gbv = pp.tile([128, 1], f32, tag=f"gbv{t}", name=f"gbv{t}")
                nc.gpsimd.indirect_dma_start(
                    out=ga[:], out_offset=None, in_=xr[:],
                    in_offset=ioa(ihx[:, t : t + 1], 0),
                )
                nc.gpsimd.indirect_dma_start(
                    out=gb[:], out_offset=None, in_=xr[:],
                    in_offset=ioa(ipx[:, t : t + 1], 0),
                )
                nc.gpsimd.indirect_dma_start(
                    out=gw[:], out_offset=None, in_=ew[:],
                    in_offset=ioa(itx[:, t : t + 1], 0),
                )
                nc.gpsimd.indirect_dma_start(
                    out=gd[:], out_offset=None, in_=dw[:],
                    in_offset=ioa(itx[:, t : t + 1], 0),
                )
                nc.gpsimd.indirect_dma_start(
                    out=gbv[:], out_offset=None, in_=db[:],
                    in_offset=ioa(itx[:, t : t + 1], 0),
                )
                GA.append(ga); GB.append(gb); GW.append(gw); GD.append(gd)
                GBV.append(gbv)

            scr = pp.tile([128, c.D], f32, tag="scr", name="scr")

            def ht_tile(nm, dt=f32, cols=HT):
                return pp.tile([128, cols], dt, tag=nm, name=nm)

            dbw, hsb, nws, dab, li0 = (
                ht_tile("dbw"), ht_tile("hsb"), ht_tile("nws"),
                ht_tile("dab"), ht_tile("li0"),
            )
            for t in range(HT):
                for acc, i0, i1 in (
                    (dbw, GB[t], GW[t]),   # h[pi] . enc_w[t]
                    (hsb, GB[t], GB[t]),   # |h[pi]|^2
                    (nws, GW[t], GW[t]),   # |enc_w[t]|^2
                    (dab, GA[t], GB[t]),   # h[i] . h[pi]
                    (li0, GA[t], GD[t]),   # h[i] . dec_w[t]
                ):
                    nc.vector.tensor_mul(out=scr[:], in0=i0[:], in1=i1[:])
                    nc.vector.reduce_sum(
                        out=acc[:, t : t + 1], in_=scr[:],
                        axis=mybir.AxisListType.X,
                    )

            ind = ht_tile("ind")
            nc.vector.tensor_scalar(
                out=ind[:], in0=dbw[:], scalar1=0.0, scalar2=None, op0=AL.is_gt
            )
            nw = ht_tile("nw")
            nc.vector.tensor_scalar_add(out=nw[:], in0=nws[:], scalar1=EPS)
            nc.scalar.sqrt(out=nw[:], in_=nw[:])
            rb = ht_tile("rb")
            nc.vector.tensor_scalar_add(out=rb[:], in0=hsb[:], scalar1=EPS)
            nc.scalar.sqrt(out=rb[:], in_=rb[:])
            nc.vector.reciprocal(out=rb[:], in_=rb[:])

            lit = ht_tile("lit")
            for t in range(HT):
                nc.vector.tensor_add(
                    out=lit[:, t : t + 1], in0=li0[:, t : t + 1], in1=GBV[t][:]
                )

            dl = ht_tile("dl")
            nc.vector.tensor_mul(out=dl[:], in0=nw[:], in1=ind[:])
            nc.vector.tensor_mul(out=dl[:], in0=dl[:], in1=dab[:])
            nc.vector.tensor_mul(out=dl[:], in0=dl[:], in1=rb[:])
            nc.vector.tensor_scalar_mul(out=dl[:], in0=dl[:], scalar1=-ALPHA)

            lpd = ht_tile("lpd")
            nc.vector.tensor_add(out=lpd[:], in0=lit[:], in1=dl[:])
            nc.sync.dma_start(out=pm(lpdh, HT), in_=lpd[:])

            # per-hit shift extras = max(0, (l+delta) - 5 - SHIFT0)
            ex = ht_tile("ex")
            nc.vector.tensor_scalar(
                out=ex[:], in0=lpd[:], scalar1=-(5.0 + SHIFT0), scalar2=0.0,
                op0=AL.add, op1=AL.max,
            )
            # scatter per-hit extras to per-row slots (separate pre-zeroed
            # tensors -> no WAW chain between the scatters)
            for t in range(HT):
                nc.gpsimd.indirect_dma_start(
                    out=exd_t[t][:], out_offset=ioa(ihc[:, t : t + 1], 0),
                    in_=ex[:, t : t + 1], in_offset=None,
                )
            # delta-s per hit: exp(l+delta-sh) - exp(l-sh).  sh = ex+SHIFT0
            # where ex is OUR extras value: only the owning core contributes a
            # row's shift extra, so the local value IS the global one and the
            # delta-s path never waits on the collective.
            t1 = ht_tile("t1")
            nc.vector.tensor_sub(out=t1[:], in0=lpd[:], in1=ex[:])
            nc.vector.tensor_scalar_add(out=t1[:], in0=t1[:], scalar1=-SHIFT0)
            nc.scalar.activation(out=t1[:], in_=t1[:], func=AF.Exp)
            t0 = ht_tile("t0")
            nc.vector.tensor_sub(out=t0[:], in0=lit[:], in1=ex[:])
            nc.vector.tensor_scalar_add(out=t0[:], in0=t0[:], scalar1=-SHIFT0)
            nc.scalar.activation(out=t0[:], in_=t0[:], func=AF.Exp)
            dsv = ht_tile("dsv")
            nc.vector.tensor_sub(out=dsv[:], in0=t1[:], in1=t0[:])
            for t in range(HT):
                nc.gpsimd.indirect_dma_start(
                    out=dsh_t[t][:], out_offset=ioa(ihc[:, t : t + 1], 0),
                    in_=dsv[:, t : t + 1], in_offset=None,
                )
            sfx = pp.tile([128, c.RT], f32, tag="sfx", name="sfx")
            dsld = [
                pp.tile([128, c.RT], f32, tag=f"dsld{t}", name=f"dsld{t}")
                for t in range(HT)
            ]
            for t in range(HT):
                nc.sync.dma_start(out=dsld[t][:], in_=pm(dsh_t[t], c.RT))
            if HT == 1:
                nc.vector.tensor_copy(out=sfx[:], in_=dsld[0][:])
            else:
                nc.vector.tensor_add(out=sfx[:], in0=dsld[0][:], in1=dsld[1][:])
                for t in range(2, HT):
                    nc.vector.tensor_add(out=sfx[:], in0=sfx[:], in1=dsld[t][:])

            exsum = pp.tile([128, c.RT], f32, tag="exsum", name="exsum")
            exld = [
                pp.tile([128, c.RT], f32, tag=f"exld{t}", name=f"exld{t}")
                for t in range(HT)
            ]
            for t in range(HT):
                nc.sync.dma_start(out=exld[t][:], in_=pm(exd_t[t], c.RT))
            if HT == 1:
                nc.vector.tensor_copy(out=exsum[:], in_=exld[0][:])
            else:
                nc.vector.tensor_add(out=exsum[:], in0=exld[0][:], in1=exld[1][:])
                for t in range(2, HT):
                    nc.vector.tensor_add(out=exsum[:], in0=exsum[:], in1=exld[t][:])
            nc.sync.dma_start(out=pm(exd, c.RT), in_=exsum[:])
            nc.gpsimd.collective_compute(
                "AllReduce", AL.add, replica_groups=groups,
                ins=[exd[:]], outs=[exg[:]],
            )
            shift = pp.tile([128, c.RT], f32, tag="shift", name="shift")
            fshift = pp.tile([128, c.RT], f32, tag="fshift", name="fshift")
            cb_sh0 = pp.tile([128, 1], f32, tag="cb_sh0", name="cb_sh0")
            nc.vector.memset(cb_sh0[:], SHIFT0)
            cb_nsh0 = pp.tile([128, 1], f32, tag="cb_nsh0", name="cb_nsh0")
            nc.vector.memset(cb_nsh0[:], -SHIFT0)
            nc.sync.dma_start(out=shift[:], in_=pm(exg, c.RT))
            nc.vector.tensor_scalar_add(out=shift[:], in0=shift[:], scalar1=SHIFT0)
            # rescale factor exp(SHIFT0 - shift): the main exp pass runs with
            # the constant SHIFT0 (so it never waits on the shift collective)
            # and the per-row sum is rescaled to the agreed shift afterwards.
            nc.scalar.activation(
                out=fshift[:], in_=shift[:], func=AF.Exp, bias=cb_sh0[:], scale=-1.0
            )

            hp_ctx.__exit__(None, None, None)

            # ---------------- phase 0: weight loads (bf16, HWDGE) ------------
            dwT_sb = [
                pp.tile([128, c.SW], bf16, tag=f"dwt{k}", name=f"dwt{k}")
                for k in range(c.KC)
            ]
            hT_sb = [
                pp.tile([128, c.NP], bf16, tag=f"ht{k}", name=f"ht{k}")
                for k in range(c.KC)
            ]
            for k in range(c.KC):
                pr = 128 if k < kc_last else pr_last
                if k == kc_last:
                    nc.gpsimd.memset(dwT_sb[k][:], 0.0)
                    nc.gpsimd.memset(hT_sb[k][:], 0.0)
                ld = (c.SW + c.LDP - 1) // c.LDP
                for j in range(c.LDP):
                    lo = j * ld
                    w = min(ld, c.SW - lo)
                    if w <= 0:
                        continue
                    nc.sync.dma_start(
                        out=dwT_sb[k][:pr, lo : lo + w],
                        in_=dwT[k * 128 : k * 128 + pr, lo : lo + w],
                    )
                nc.sync.dma_start(
                    out=hT_sb[k][:pr, :], in_=xT[k * 128 : k * 128 + pr, :]
                )

            # ---------------- main pipeline ----------------------------------
            s_all = pp.tile([128, c.RT], f32, tag="s_all", name="s_all")
            nlz = pp.tile([128, c.RT], f32, tag="nlz", name="nlz")
            maxchunk = max(c.chunk)

            # PSUM groups: PSG col-tiles share one wide PSUM tile so the
            # psum->sbuf copy runs as one wide DVE instruction.
            groups_n = []
            n = 0
            while n < c.CT:
                g = min(c.PSG, c.CT - n)
                groups_n.append((n, g))
                n += g
            expw = (c.SW + c.EXPW - 1) // c.EXPW

            m_starts = []
            m0 = 0
            for csz in c.chunk:
                m_starts.append(m0)
                m0 += csz
            LGs: dict[int, list] = {}

            def emit_compute(ci):
                csz, m0 = c.chunk[ci], m_starts[ci]
                LG = [
                    pp.tile(
                        [128, c.SW], bf16, tag=f"lg{mi}", name=f"lg{ci}_{mi}",
                        bufs=c.LGB,
                    )
                    for mi in range(csz)
                ]
                LGs[ci] = LG
                for mi in range(csz):
                    m = m0 + mi
                    for n0, g in groups_n:
                        ps = psp.tile(
                            [128, g * c.MT], f32, tag=f"ps{g}",
                            name=f"ps{ci}_{mi}_{n0}",
                        )
                        for k in range(c.KC):
                            for j in range(g):
                                lo = (n0 + j) * c.MT
                                mw = min(c.MT, c.SW - lo)
                                if mw <= 0:
                                    continue
                                nc.tensor.matmul(
                                    ps[:, j * c.MT : j * c.MT + mw],
                                    lhsT=hT_sb[k][:, m * 128 : (m + 1) * 128],
                                    rhs=dwT_sb[k][:, lo : lo + mw],
                                    start=(k == 0),
                                    stop=(k == c.KC - 1),
                                )
                        cw = min((n0 + g) * c.MT, c.SW) - n0 * c.MT
                        nc.vector.tensor_copy(
                            out=LG[mi][:, n0 * c.MT : n0 * c.MT + cw],
                            in_=ps[:, :cw],
                        )
                prt = [
                    pp.tile([128, c.EXPW], f32, tag=f"pt{mi}", name=f"pt{ci}_{mi}")
                    for mi in range(csz)
                ]
                for mi in range(csz):
                    m = m0 + mi
                    for e in range(c.EXPW):
                        lo = e * expw
                        w = min(expw, c.SW - lo)
                        if w <= 0:
                            continue
                        es = esp.tile(
                            [128, expw], bf16, tag="es", name=f"es{ci}_{mi}_{e}"
                        )
                        nc.scalar.activation(
                            out=es[:, :w], in_=LG[mi][:, lo : lo + w], func=AF.Exp,
                            bias=cb_nsh0[:], scale=1.0,
                            accum_out=prt[mi][:, e : e + 1],
                        )
                    nc.vector.reduce_sum(
                        out=s_all[:, m : m + 1], in_=prt[mi][:],
                        axis=mybir.AxisListType.X,
                    )
                msl = slice(m0, m0 + csz)
                sin = pp.tile(
                    [128, maxchunk], f32, tag="sin", name=f"sin{ci}", bufs=2
                )
                nc.vector.tensor_mul(
                    out=sin[:, :csz], in0=s_all[:, msl], in1=fshift[:, msl]
                )
                nc.vector.tensor_add(
                    out=sin[:, :csz], in0=sin[:, :csz], in1=sfx[:, msl]
                )
                sins[ci] = sin

            def emit_collective(ci):
                csz = c.chunk[ci]
                # sin tile of chunk ci is the latest allocation of tag "sin"
                nc.sync.dma_start(out=ccin[ci][:], in_=sins[ci][:, :csz])
                nc.gpsimd.collective_compute(
                    "AllReduce", AL.add, replica_groups=groups,
                    ins=[ccin[ci][:]], outs=[ccout[ci][:]],
                )

            def emit_finish(ci):
                csz, m0 = c.chunk[ci], m_starts[ci]
                msl = slice(m0, m0 + csz)
                LG = LGs.pop(ci)
                sg = pp.tile(
                    [128, maxchunk], f32, tag="sg", name=f"sg{ci}", bufs=2
                )
                nc.sync.dma_start(out=sg[:, :csz], in_=ccout[ci][:])
                nc.scalar.activation(out=sg[:, :csz], in_=sg[:, :csz], func=AF.Ln)
                nc.vector.tensor_add(
                    out=sg[:, :csz], in0=sg[:, :csz], in1=shift[:, msl]
                )
                nc.vector.tensor_scalar_mul(
                    out=nlz[:, msl], in0=sg[:, :csz], scalar1=-1.0
                )
                # final pass (in place on the bf16 logits, one wide op per
                # row-tile), then one casting DMA out (bf16 -> f32) per
                # row-tile so each output DMA waits only on its writer
                for mi in range(csz):
                    m = m0 + mi
                    r0 = m * 128
                    rp = min(128, c.N - r0)
                    if rp <= 0:
                        continue
                    if (m % max(1, int(round(1 / max(c.act_frac, 1e-6))))) == 0:
                        nc.scalar.add(
                            out=LG[mi][:rp, : c.SW], in_=LG[mi][:rp, : c.SW],
                            add=nlz[:rp, m : m + 1],
                        )
                    else:
                        nc.vector.tensor_scalar_add(
                            out=LG[mi][:rp, : c.SW], in0=LG[mi][:rp, : c.SW],
                            scalar1=nlz[:rp, m : m + 1],
                        )
                    nc.gpsimd.dma_start(
                        out=out_ext[r0 : r0 + rp, :], in_=LG[mi][:rp, : c.SW]
                    )

            # software-pipelined emission: chunk ci's compute and collective
            # are emitted before chunk ci-1's normalize/final/store, so no
            # sequencer sits in a wait that blocks the next chunk's launch.
            sins: dict[int, object] = {}
            NCH = len(c.chunk)
            lag = max(1, min(c.FINLAG, c.LGB - 1))
            for ci in range(NCH):
                emit_compute(ci)
                if ci >= lag:
                    emit_finish(ci - lag)
                emit_collective(ci)
            for ci in range(max(0, NCH - lag), NCH):
                emit_finish(ci)

            nc.sync.dma_start(out=pm(nlzd, c.RT), in_=nlz[:])

    # ------------- second context: patch hit elements of the output ---------
    # A fresh TileContext begins after the first context's full drain, so the
    # indirect scatters carry no conservative WAW deps against the main
    # output DMAs.  Between the per-tile scatters, 8 dummy reads flush the
    # previous scatter's tick to every DMASW proc (DMA wait limit is 1).
    with tile.TileContext(nc) as tc2:
        with tc2.tile_pool(name="patch", bufs=1) as qq:
            ih2 = qq.tile([128, HT], i32, tag="ih2", name="ih2")
            io2r = qq.tile([128, HT], i32, tag="io2r", name="io2r")
            lp2 = qq.tile([128, HT], f32, tag="lp2", name="lp2")
            nc.sync.dma_start(out=ih2[:], in_=pm(hh, HT))
            nc.sync.dma_start(out=io2r[:], in_=pm(hoff, HT))
            nc.sync.dma_start(out=lp2[:], in_=pm(lpdh, HT))
            io2 = qq.tile([128, HT], i32, tag="io2", name="io2")
            nc.vector.tensor_copy(out=io2[:], in_=io2r[:])
            lzv = qq.tile([128, HT], f32, tag="lzv", name="lzv")
            for t in range(HT):
                nc.gpsimd.indirect_dma_start(
                    out=lzv[:, t : t + 1], out_offset=None, in_=nlzd[:],
                    in_offset=ioa(ih2[:, t : t + 1], 0),
                )
            vv = qq.tile([128, HT], f32, tag="vv", name="vv")
            nc.vector.tensor_add(out=vv[:], in0=lp2[:], in1=lzv[:])
            dum2 = qq.tile([128, 8 * HT], f32, tag="dum2", name="dum2")
            for t in range(HT):
                nc.gpsimd.indirect_dma_start(
                    out=out_flat, out_offset=ioa(io2[:, t : t + 1], 0),
                    in_=vv[:, t : t + 1], in_offset=None,
                )
                if t + 1 < HT:
                    for q in range(8):
                        nc.gpsimd.dma_start(
                            out=dum2[0:1, t * 8 + q : t * 8 + q + 1],
                            in_=out_ext[q : q + 1, 0:1],
                        )

    nc.compile()
    return nc


def prepare(cfg: Cfg, x, dec_w, dec_b, enc_w, targets):
    """Host-side sharding / index prep (numpy). Returns (in_maps, widths, maxh)."""
    c = cfg
    x2 = np.ascontiguousarray(np.asarray(x, dtype=np.float32).reshape(-1, c.D))
    dec_w = np.asarray(dec_w, dtype=np.float32)
    dec_b = np.asarray(dec_b, dtype=np.float32).reshape(-1)
    enc_w = np.asarray(enc_w, dtype=np.float32)
    t = np.asarray(targets).astype(np.int64).reshape(-1)
    assert x2.shape == (c.N, c.D) and t.shape == (c.N,)

    import ml_dtypes

    bf = ml_dtypes.bfloat16
    # row 0 is the "ones" row so the bias lands on partition 0 of k-chunk 0
    # (partition-sliced memsets must start at a 32-aligned partition).
    xT = np.zeros((c.K, c.NP), bf)
    xT[0, :] = bf(1.0)
    xT[1:, : c.N] = x2.T.astype(bf)

    owner = np.minimum(t // c.SW, c.NC - 1)
    tl = (t - owner * c.SW).astype(np.int64)
    # last-occurrence map (scatter-then-gather semantics of the reference)
    last = {}
    for j in range(c.N):
        last[int(t[j])] = j
    pi = np.array([last[int(v)] for v in t], dtype=np.int64)

    counts = [int(np.sum(owner == ci)) for ci in range(c.NC)]
    assert min(counts) > 0, "a core has zero hits; SPMD clone-padding needs >=1"
    maxh = ((max(counts) + 127) // 128) * 128
    HT = maxh // 128

    zrow = np.zeros((c.NP, 1), np.float32)
    in_maps = []
    widths = []
    for ci in range(c.NC):
        lo = ci * c.SW
        hi = min(lo + c.SW, c.V)
        w = hi - lo
        widths.append(w)

        dwT_h = np.zeros((c.K, c.SW), bf)
        dwT_h[0, :w] = dec_b[lo:hi].astype(bf)
        dwT_h[1:, :w] = dec_w[lo:hi].T.astype(bf)
        if w < c.SW:
            dwT_h[0, w:] = bf(PAD_BIAS)

        dw_h = np.zeros((c.SW, c.D), np.float32)
        dw_h[:w] = dec_w[lo:hi]
        ew_h = np.zeros((c.SW, c.D), np.float32)
        ew_h[:w] = enc_w[lo:hi]
        db_h = np.zeros((c.SW, 1), np.float32)
        db_h[:w, 0] = dec_b[lo:hi]

        rows = np.nonzero(owner == ci)[0]
        pad = np.full(maxh - len(rows), rows[0], dtype=np.int64)
        rows_p = np.concatenate([rows, pad])
        hh_h = rows_p.astype(np.int32).reshape(-1, 1)
        hp_h = pi[rows_p].astype(np.int32).reshape(-1, 1)
        htl_h = tl[rows_p].astype(np.int32).reshape(-1, 1)
        hoff_h = (rows_p * c.SW + tl[rows_p]).astype(np.int32).reshape(-1, 1)

        im = {
            "xT": xT,
            "x": x2,
            "dwT": dwT_h,
            "dw": dw_h,
            "ew": ew_h,
            "db": db_h,
            "hh": hh_h,
            "hp": hp_h,
            "htl": htl_h,
            "hoff": hoff_h,
        }
        for tt in range(HT):
            im[f"exd{tt}"] = zrow
            im[f"dsh{tt}"] = zrow
        in_maps.append(im)
    return in_maps, widths, maxh


def run(inputs: dict, cfg: Cfg | None = None, trace: bool = False):
    cfg = cfg or Cfg()
    in_maps, widths, maxh = prepare(
        cfg,
        inputs["x"],
        inputs["dec_w"],
        inputs["dec_b"],
        inputs["enc_w"],
        inputs["targets"],
    )
    nc = build(cfg, maxh)
    bkr = run_bass_kernel_spmd(nc, in_maps, list(range(cfg.NC)), trace=trace)
    res = bkr.results
    out = np.concatenate(
        [res[ci]["out"][:, : widths[ci]] for ci in range(cfg.NC)], axis=1
    )
    return np.ascontiguousarray(out, dtype=np.float32), bkr


def kernel(x, dec_w, dec_b, enc_w, targets):
    out, _ = run(
        {"x": x, "dec_w": dec_w, "dec_b": dec_b, "enc_w": enc_w, "targets": targets}
    )
    return out
